# revision 1
# baseline (speedup 1.0000x reference)
"""Trainium2 Bass kernel for nn_Covar_Attn (MPNCOV-style covariance pooling).

Per sample s (of 32): X = x[s] viewed [C=512, M=784]
  cov  = (X-mu) @ (X-mu)^T / M                  [512, 512]
  A    = cov / trace(cov)
  Ysqrt= Newton-Schulz(A, 5 iters) * sqrt(trace)
  w    = mean over rows of Ysqrt                [512]
  y[s] = w[:, None] * X

Sharding: pure data parallel, 4 samples per NeuronCore across 8 cores.

All matmuls run in float32r (TF32-like, 1 cycle/row at N>=256 vs 4 for fp32).
Every Newton-Schulz iterate is a polynomial of the symmetric matrix A, hence
symmetric, so lhsT == the matrix itself (no transposes needed inside NS).
The Ysqrt row-mean is computed with row-vector chains (no full Y4/Zs4/Ysqrt
products). Samples are processed in braided pairs so one sample's matmuls
fill the other's formation/copy stalls.
"""

import numpy as np
from contextlib import ExitStack

import concourse.bass as bass
import concourse.mybir as mybir
import concourse.tile as tile
from concourse import bacc
from concourse.bass_utils import run_bass_kernel_spmd

N_CORES = 8
B, C, H, W = 32, 512, 28, 28
M = H * W            # 784
B_LOC = B // N_CORES  # 4 samples per core
CCH = C // 128       # 4 chunks of 128 rows
MCH = 7              # m chunks
MC = M // MCH        # 112
ITER_N = 5

F32 = mybir.dt.float32
F32R = mybir.dt.float32r
MULT = mybir.AluOpType.mult
ADD = mybir.AluOpType.add
SUB = mybir.AluOpType.subtract
AX = mybir.AxisListType.X


def _fill_diag(nc, t, val):
    nc.gpsimd.memset(t[:], 0.0)
    nc.gpsimd.affine_select(
        out=t[:],
        in_=t[:],
        compare_op=mybir.AluOpType.not_equal,
        fill=val,
        base=0,
        pattern=[[-1, 128]],
        channel_multiplier=1,
    )


class _Emit:
    @staticmethod
    def _w(i):
        # computed width of chunk-row i (>=256 keeps f32r at 1 cyc/row)
        return max(C - i * 128, 256)

    def __init__(self, ctx, tc, x_ap, y_ap):
        nc = self.nc = tc.nc
        self.tc = tc
        p = lambda name, bufs, **kw: ctx.enter_context(
            tc.tile_pool(name=name, bufs=bufs, **kw)
        )
        self.consts = p("consts", 1)
        self.xin_p = p("xin", 4)
        self.xt_p = p("xt", 2)
        self.an_p = p("an", 2)
        self.y_p = p("yy", 3)
        self.zy_p = p("zy", 3)
        self.zs_p = p("zs", 3)
        self.sm_p = p("sm", 2)
        self.ps_mm = p("psmm", 3, space="PSUM")
        self.ps_tr = p("pstr", 2, space="PSUM")
        self.ps_sm = p("pssm", 3, space="PSUM")

        ident = self.ident = self.consts.tile([128, 128], F32, tag="ident", name="ident")
        _fill_diag(nc, ident, 1.0)
        self.i075 = self.consts.tile([128, 128], F32, tag="i075", name="i075")
        _fill_diag(nc, self.i075, 0.75)
        self.i15 = self.consts.tile([128, 128], F32, tag="i15", name="i15")
        _fill_diag(nc, self.i15, 1.5)
        ones_f = self.ones_f = self.consts.tile([128, 128], F32, tag="ones_f", name="ones_f")
        nc.gpsimd.memset(ones_f[:], 1.0)
        self.ones_r = self.consts.tile([128, 128], F32R, tag="ones_r", name="ones_r")
        nc.vector.tensor_copy(self.ones_r[:], ones_f[:])
        self.ones_col = self.consts.tile([128, CCH], F32R, tag="onec", name="onec")
        nc.vector.tensor_copy(self.ones_col[:], ones_f[:, 0:CCH])
        self.ident_r = self.consts.tile([128, 128], F32R, tag="ident_r", name="ident_r")
        nc.vector.tensor_copy(self.ident_r[:], ident[:])

        self.xr = x_ap.rearrange("b (i p) m -> b p i m", p=128)
        self.yr = y_ap.rearrange("b (i p) m -> b p i m", p=128)
        self.S = [dict() for _ in range(B_LOC)]

    # ---------- phases ----------
    def load(self, s):
        nc, st = self.nc, self.S[s]
        x_t = st["x"] = self.xin_p.tile([128, CCH, M], F32, tag="x", name="x")
        for i in range(CCH):
            nc.sync.dma_start(x_t[:, i, :], self.xr[s, :, i, :])
        stt = self.sm_p.tile([128, CCH, 2, 6], F32, tag="st", name="st")
        for i in range(CCH):
            for h in range(2):
                nc.vector.bn_stats(
                    stt[:, i, h, :], x_t[:, i, h * (M // 2):(h + 1) * (M // 2)]
                )
        mv = st["mv"] = self.sm_p.tile([128, CCH, 2], F32, tag="mv", bufs=4, name="mv")
        for i in range(CCH):
            nc.vector.bn_aggr(mv[:, i, :], stt[:, i, :, :])
        for i in range(CCH):
            nc.gpsimd.tensor_scalar_sub(x_t[:, i, :], x_t[:, i, :], mv[:, i, 0:1])
        # trace(cov) = sum_c var_c (broadcast via ones matmul)
        var_r = self.sm_p.tile([128, CCH], F32R, tag="var_r", name="var_r")
        nc.vector.tensor_copy(var_r[:], mv[:, :, 1])
        t_ps = self.ps_sm.tile([128, CCH], F32, tag="sm", name="sm")
        nc.tensor.matmul(t_ps[:], self.ones_r[:], var_r[:], start=True, stop=True)
        tco = self.sm_p.tile([128, 1], F32, tag="tco", name="tco")
        nc.vector.reduce_sum(out=tco[:], in_=t_ps[:], axis=AX)
        inv = st["inv"] = self.sm_p.tile([128, 1], F32, tag="inv", name="inv")
        nc.vector.reciprocal(inv[:], tco[:])
        sq = st["sq"] = self.sm_p.tile([128, 1], F32, tag="sq", bufs=4, name="sq")
        nc.scalar.sqrt(sq[:], tco[:])

    def trans(self, s, j):
        nc, st = self.nc, self.S[s]
        if j == 0:
            st["xt"] = self.xt_p.tile([MC, MCH, C], F32R, tag="xt", name="xt")
        xt, xc = st["xt"], st["x"]
        for i in range(CCH):
            tp = self.ps_tr.tile([MC, 128], F32, tag="tr", name="tr")
            nc.tensor.transpose(tp[:], xc[:, i, j * MC:(j + 1) * MC], self.ident[:])
            nc.scalar.copy(xt[:, j, i * 128:(i + 1) * 128], tp[:])

    def cov(self, s, i):
        nc, st = self.nc, self.S[s]
        if i == 0:
            st["an"] = self.an_p.tile([128, CCH, C], F32R, tag="An", name="An")
        xt, an = st["xt"], st["an"]
        w = self._w(i)
        g = self.ps_mm.tile([128, C], F32, tag="mm", name="mm")
        for j in range(MCH):
            nc.tensor.matmul(
                g[:, 0:w], xt[:, j, i * 128:(i + 1) * 128], xt[:, j, C - w:],
                start=(j == 0), stop=(j == MCH - 1),
            )
        nc.vector.tensor_scalar(
            an[:, i, C - w:], g[:, 0:w], st["inv"][:], 1.0 / M, op0=MULT, op1=MULT
        )
        self._mirror(an, i)

    def _mirror(self, mat_t, i):
        nc = self.nc
        for k in range(i + 1, CCH):
            if i * 128 >= C - self._w(k):
                continue
            tp = self.ps_tr.tile([128, 128], F32R, tag="tr", name="tr")
            nc.tensor.transpose(
                tp[:], mat_t[:, i, k * 128:(k + 1) * 128], self.ident_r[:]
            )
            nc.scalar.copy(mat_t[:, k, i * 128:(i + 1) * 128], tp[:].bitcast(F32))

    def iter1_zy(self, s):
        nc, st = self.nc, self.S[s]
        zy = st["zs"] = self.zs_p.tile([128, CCH, C], F32R, tag="zs", name="zs1")
        an = st["an"]
        for i in range(CCH):
            eng = nc.vector if i % 2 == 0 else nc.scalar
            if eng is nc.vector:
                nc.vector.tensor_scalar_mul(zy[:, i, :], an[:, i, :].bitcast(F32), -0.25)
            else:
                nc.scalar.mul(zy[:, i, :], an[:, i, :].bitcast(F32), -0.25)
            nc.gpsimd.tensor_tensor(
                zy[:, i, i * 128:(i + 1) * 128],
                zy[:, i, i * 128:(i + 1) * 128].bitcast(F32),
                self.i075[:], op=ADD,
            )

    def iter1_y(self, s, i):
        nc, st = self.nc, self.S[s]
        if i == 0:
            st["y"] = self.y_p.tile([128, CCH, C], F32R, tag="Y", name="Y")
        an, zy, y_c = st["an"], st["zs"], st["y"]
        w = self._w(i)
        ps = self.ps_mm.tile([128, C], F32, tag="mm", name="mm")
        for k in range(CCH):
            nc.tensor.matmul(
                ps[:, 0:w], an[:, k, i * 128:(i + 1) * 128], zy[:, k, C - w:],
                start=(k == 0), stop=(k == CCH - 1),
            )
        nc.scalar.mul(y_c[:, i, C - w:], ps[:, 0:w], 2.0)
        self._mirror(y_c, i)

    def prod_T(self, s, i, last):
        """T = Zs @ Y -> ZY = 1.5I - T (chunk i)."""
        nc, st = self.nc, self.S[s]
        if i == 0:
            st["zyn"] = self.zy_p.tile([128, CCH, C], F32R, tag="zy", name="zy")
        zs_c, y_c, zyn = st["zs"], st["y"], st["zyn"]
        w = self._w(i)
        ps = self.ps_mm.tile([128, C], F32, tag="mm", name="mm")
        for k in range(CCH):
            nc.tensor.matmul(
                ps[:, 0:w], zs_c[:, k, i * 128:(i + 1) * 128], y_c[:, k, C - w:],
                start=(k == 0), stop=(k == CCH - 1),
            )
        nc.scalar.mul(zyn[:, i, C - w:], ps[:, 0:w], -1.0)
        nc.gpsimd.tensor_tensor(
            zyn[:, i, i * 128:(i + 1) * 128],
            zyn[:, i, i * 128:(i + 1) * 128].bitcast(F32),
            self.i15[:], op=ADD,
        )
        self._mirror(zyn, i)

    def prod_Y(self, s, i):
        nc, st = self.nc, self.S[s]
        if i == 0:
            st["yn"] = self.y_p.tile([128, CCH, C], F32R, tag="Y", name="Y")
        y_c, zyn, yn = st["y"], st["zyn"], st["yn"]
        w = self._w(i)
        ps = self.ps_mm.tile([128, C], F32, tag="mm", name="mm")
        for k in range(CCH):
            nc.tensor.matmul(
                ps[:, 0:w], y_c[:, k, i * 128:(i + 1) * 128], zyn[:, k, C - w:],
                start=(k == 0), stop=(k == CCH - 1),
            )
        nc.scalar.copy(yn[:, i, C - w:], ps[:, 0:w])
        self._mirror(yn, i)

    def prod_Z(self, s, i):
        nc, st = self.nc, self.S[s]
        if i == 0:
            st["zsn"] = self.zs_p.tile([128, CCH, C], F32R, tag="zs", name="zs")
        zs_c, zyn, zsn = st["zs"], st["zyn"], st["zsn"]
        w = self._w(i)
        ps = self.ps_mm.tile([128, C], F32, tag="mm", name="mm")
        for k in range(CCH):
            nc.tensor.matmul(
                ps[:, 0:w], zyn[:, k, i * 128:(i + 1) * 128], zs_c[:, k, C - w:],
                start=(k == 0), stop=(k == CCH - 1),
            )
        nc.scalar.copy(zsn[:, i, C - w:], ps[:, 0:w])
        self._mirror(zsn, i)
        if i == CCH - 1:
            st["y"], st["zs"] = st["yn"], st["zsn"]

    # ---- vectorized tail ----
    def _row_mvm(self, col_r, mat_t):
        nc = self.nc
        pr = self.ps_sm.tile([1, C], F32, tag="sm", name="sm")
        for k in range(CCH):
            nc.tensor.matmul(
                pr[:], col_r[:, k:k + 1], mat_t[:, k, :],
                start=(k == 0), stop=(k == CCH - 1),
            )
        return pr

    def _row_to_col(self, row_ps, tag):
        nc = self.nc
        r_sb = self.sm_p.tile([1, C], F32, tag="r_sb", bufs=3, name="r_sb")
        nc.scalar.copy(r_sb[:], row_ps[:])
        tp = self.ps_sm.tile([128, CCH], F32, tag="sm", name="sm")
        for k in range(CCH):
            nc.tensor.transpose(
                tp[:, k:k + 1], r_sb[0:1, k * 128:(k + 1) * 128],
                self.ident[0:1, 0:1],
            )
        col = self.sm_p.tile([128, CCH], F32R, tag=tag + "_c", name="tile")
        nc.scalar.copy(col[:], tp[:])
        return col

    def tail_steps(self, s):
        nc, st = self.nc, self.S[s]
        # w_row = 1.5 * (1^T Y4) - 1^T Y4 Zs4 Y4
        #       = 1.5 v - ((((v ZY4) Zs3) Y3) ZY4),  v = (1^T Y3) ZY4
        y3, zs3, zy4 = st["y"], st["zs"], st["zyn"]
        a_ps = self._row_mvm(self.ones_col, y3)
        yield
        a_c = self._row_to_col(a_ps, "a")
        yield
        v_ps = self._row_mvm(a_c, zy4)
        yield
        v_sb = self.sm_p.tile([1, C], F32, tag="v_sb", name="v_sb")
        nc.vector.tensor_scalar_mul(v_sb[:], v_ps[:], 1.5)
        v_c = self._row_to_col(v_ps, "v")
        yield
        d1_c = self._row_to_col(self._row_mvm(v_c, zy4), "d1")
        yield
        d2_c = self._row_to_col(self._row_mvm(d1_c, zs3), "d2")
        yield
        d3_c = self._row_to_col(self._row_mvm(d2_c, y3), "d3")
        yield
        u_ps = self._row_mvm(d3_c, zy4)
        w_row = self.sm_p.tile([1, C], F32, tag="w_row", name="w_row")
        nc.vector.tensor_tensor(w_row[:], v_sb[:], u_ps[:], op=SUB)
        yield
        wt_ps = self.ps_sm.tile([128, CCH], F32, tag="sm", name="sm")
        for k in range(CCH):
            nc.tensor.transpose(
                wt_ps[:, k:k + 1], w_row[0:1, k * 128:(k + 1) * 128],
                self.ident[0:1, 0:1],
            )
        fs = st["fs"] = self.sm_p.tile([128, CCH], F32, tag="fs", name="fs")
        nc.vector.tensor_scalar(fs[:], wt_ps[:], st["sq"][:], 1.0 / C, op0=MULT, op1=MULT)

    def fin(self, s):
        nc, st = self.nc, self.S[s]
        x_t, mv, fs = st["x"], st["mv"], st["fs"]
        mufs = self.sm_p.tile([128, CCH], F32, tag="mufs", name="mufs")
        nc.vector.tensor_tensor(mufs[:], mv[:, :, 0], fs[:], op=MULT)
        for i in range(CCH):
            eng = nc.vector if i % 2 == 0 else nc.gpsimd
            eng.tensor_scalar(
                x_t[:, i, :], x_t[:, i, :], fs[:, i:i + 1], mufs[:, i:i + 1],
                op0=MULT, op1=ADD,
            )
        nc.sync.dma_start(self.yr[s], x_t[:])
        st.clear()

    def transcov_gen(self, pair):
        for j in range(MCH):
            for s in pair:
                self.trans(s, j)
            yield
        for i in range(CCH):
            for s in pair:
                self.cov(s, i)
            yield

    def ns_pair(self, pair):
        for s in pair:
            self.iter1_zy(s)
        for i in range(CCH):
            for s in pair:
                self.iter1_y(s, i)
        for it in range(ITER_N - 3):
            for i in range(CCH):
                for s in pair:
                    self.prod_T(s, i, last=False)
            for s in pair:
                for i in range(CCH):
                    self.prod_Y(s, i)
            for s in pair:
                for i in range(CCH):
                    self.prod_Z(s, i)
        for i in range(CCH):
            for s in pair:
                self.prod_T(s, i, last=True)

    @staticmethod
    def _round_robin(gens):
        done = [False] * len(gens)
        while not all(done):
            for gi, g in enumerate(gens):
                if not done[gi]:
                    try:
                        next(g)
                    except StopIteration:
                        done[gi] = True


def _emit(ctx, tc, x_ap, y_ap):
    em = _Emit(ctx, tc, x_ap, y_ap)
    em.load(0)
    em.load(1)
    em._round_robin([em.transcov_gen((0, 1))])
    em.ns_pair((0, 1))
    em.load(2)
    em.load(3)
    em._round_robin([em.tail_steps(0), em.tail_steps(1), em.transcov_gen((2, 3))])
    em.fin(0)
    em.fin(1)
    em.ns_pair((2, 3))
    em._round_robin([em.tail_steps(2), em.tail_steps(3)])
    em.fin(2)
    em.fin(3)


_NC_CACHE = {}


def _get_nc(reps: int = 1):
    if reps not in _NC_CACHE:
        nc = bacc.Bacc("TRN2", target_bir_lowering=False, debug=False)
        x_ap = nc.dram_tensor("x", [B_LOC, C, M], F32, kind="ExternalInput").ap()
        y_ap = nc.dram_tensor("y", [B_LOC, C, M], F32, kind="ExternalOutput").ap()
        with ExitStack() as ctx:
            tc = ctx.enter_context(tile.TileContext(nc))
            if reps > 1:
                with tc.For_i(0, reps, 1):
                    _emit(ctx, tc, x_ap, y_ap)
            else:
                _emit(ctx, tc, x_ap, y_ap)
        nc.compile()
        _NC_CACHE[reps] = nc
    return _NC_CACHE[reps]


def kernel(x: np.ndarray, _trace: bool = False):
    assert x.shape == (B, C, H, W), x.shape
    xs = np.ascontiguousarray(x.reshape(B, C, M), dtype=np.float32)
    nc = _get_nc()
    in_maps = [
        {"x": np.ascontiguousarray(xs[c * B_LOC:(c + 1) * B_LOC])}
        for c in range(N_CORES)
    ]
    res = run_bass_kernel_spmd(nc, in_maps, core_ids=list(range(N_CORES)), trace=_trace)
    y = np.concatenate([res.results[c]["y"] for c in range(N_CORES)], axis=0)
    out = y.reshape(B, C, H, W).astype(np.float32)
    if _trace:
        return out, res
    return out



# revision 8
# speedup vs baseline: 1.4955x; 1.4955x over previous
"""Trainium2 Bass kernel for nn_Covar_Attn (MPNCOV-style covariance pooling).

Per sample s (of 32): X = x[s] viewed [C=512, M=784]
  cov  = (X-mu) @ (X-mu)^T / M                  [512, 512]
  A    = cov / trace(cov)
  Ysqrt= Newton-Schulz(A, 5 iters) * sqrt(trace)
  w    = mean over rows of Ysqrt                [512]
  y[s] = w[:, None] * X

Sharding: pure data parallel, 4 samples per NeuronCore across 8 cores.

v2 design:
- All matmul operands bf16 (1 cyc/col at any width; FWL weight loads on HW).
- No centering pass: cov accumulated from raw x with a rank-1 -M*mu*mu^T
  matmul folded into the same PSUM; x stays raw so the final scale is one op.
- W-iteration Newton-Schulz: with W_k = Z_k Y_k (= A Z_k^2),
  W_{k+1} = (9 W - 6 W^2 + W^3)/4 needs 2 matrix products per iteration
  (vs 3 for the Y/Z form); the 9W and -6W^2 terms are accumulated into the
  W^3 PSUM via constant-diagonal matmuls.
- 1^T Ysqrt = 1^T An ZY0 ZY1 ZY2 ZY3 ZY4 with v@ZY_k = 1.5v - 0.5 v@W_k and
  v@W4 expanded into three extra W3 matvecs. The first two steps come free
  from accum_out rowsums of the An and S1 copy-outs; the remaining 6 matvec
  steps run as row-mode chains on the PE.
- All matrices symmetric: only upper-triangle chunk-rows are computed
  (true-triangle widths 512/384/256/128); lower blocks are mirrored with PE
  transposes and destination-contiguous batched copies.
- PSUM->SBUF copies alternate between Activation and DVE (gpsimd has no
  PSUM port); gpsimd takes SBUF-resident work (final scaling, combines).
"""

import numpy as np
from contextlib import ExitStack

import concourse.bass as bass
import concourse.mybir as mybir
import concourse.tile as tile
from concourse import bacc
from concourse.bass_utils import run_bass_kernel_spmd

N_CORES = 8
B, C, H, W = 32, 512, 28, 28
M = H * W            # 784
B_LOC = B // N_CORES  # 4 samples per core
CCH = C // 128       # 4 chunks of 128 rows
MCH = 7              # m chunks
MC = M // MCH        # 112

F32 = mybir.dt.float32
F32R = mybir.dt.float32r
BF16 = mybir.dt.bfloat16
MULT = mybir.AluOpType.mult
ADD = mybir.AluOpType.add
SUB = mybir.AluOpType.subtract
AX = mybir.AxisListType.X
COPYF = mybir.ActivationFunctionType.Copy


def _wid(i):
    # true-triangle width of chunk-row i (diag block + right part)
    return C - i * 128


def _fill_diag(nc, t, val):
    nc.gpsimd.memset(t[:], 0.0)
    nc.gpsimd.affine_select(
        out=t[:],
        in_=t[:],
        compare_op=mybir.AluOpType.not_equal,
        fill=val,
        base=0,
        pattern=[[-1, 128]],
        channel_multiplier=1,
    )


class _Emit:
    def __init__(self, ctx, tc, x_ap, y_ap):
        nc = self.nc = tc.nc
        self.tc = tc
        p = lambda name, bufs, **kw: ctx.enter_context(
            tc.tile_pool(name=name, bufs=bufs, **kw)
        )
        self.consts = p("consts", 1)
        self.xin_p = p("xin", 4)
        self.xt_p = p("xt", 2)
        self.w_p = p("wmat", 8)
        self.s_p = p("smat", 3)
        self.sm_p = p("sm", 2)
        self.ps_mm = p("psmm", 2, space="PSUM")
        self.ps_tr = p("pstr", 2, space="PSUM")
        self.ps_mr = p("psmr", 2, space="PSUM")
        self.ps_sm = p("pssm", 1, space="PSUM")

        # constants
        identf = self.identf = self.consts.tile([128, 128], F32, tag="identf", name="identf")
        _fill_diag(nc, identf, 1.0)
        self.ident_b = self.consts.tile([128, 128], BF16, tag="ident_b", name="ident_b")
        nc.vector.tensor_copy(self.ident_b[:], identf[:])
        d9 = self.consts.tile([128, 128], F32, tag="d9f", name="d9f")
        _fill_diag(nc, d9, 9.0)
        self.diag9 = self.consts.tile([128, 128], BF16, tag="diag9", name="diag9")
        nc.vector.tensor_copy(self.diag9[:], d9[:])
        dm6 = self.consts.tile([128, 128], F32, tag="dm6f", name="dm6f")
        _fill_diag(nc, dm6, -6.0)
        self.diagm6 = self.consts.tile([128, 128], BF16, tag="diagm6", name="diagm6")
        nc.vector.tensor_copy(self.diagm6[:], dm6[:])
        onesf = self.consts.tile([128, 128], F32, tag="onesf", name="onesf")
        nc.gpsimd.memset(onesf[:], 1.0)
        self.ones_b = self.consts.tile([128, 128], BF16, tag="ones_b", name="ones_b")
        nc.vector.tensor_copy(self.ones_b[:], onesf[:])

        self.xr = x_ap.rearrange("b (i p) m -> b p i m", p=128)
        self.yr = y_ap.rearrange("b (i p) m -> b p i m", p=128)
        self.S = [dict() for _ in range(B_LOC)]
        self._cp_i = 0

    def _cp(self, out, in_, scale=None):
        """PSUM->SBUF copy alternating between Activation and DVE."""
        use_act = (self._cp_i % 2 == 0)
        self._cp_i += 1
        if use_act:
            if scale is None:
                self.nc.scalar.copy(out, in_)
            else:
                self.nc.scalar.mul(out, in_, scale)
        else:
            if scale is None:
                self.nc.vector.tensor_copy(out, in_)
            else:
                self.nc.vector.tensor_scalar_mul(out, in_, scale)

    # ---------- load & stats ----------
    def dma_in(self, s):
        nc, st = self.nc, self.S[s]
        x_t = st["x"] = self.xin_p.tile([128, CCH, M], F32, tag="x", name="x")
        for i in range(CCH):
            nc.sync.dma_start(x_t[:, i, :], self.xr[s, :, i, :])

    def stats(self, s):
        nc, st = self.nc, self.S[s]
        x_t = st["x"]
        stt = self.sm_p.tile([128, CCH, 2, 6], F32, tag="st", name="st")
        for i in range(CCH):
            for h in range(2):
                nc.vector.bn_stats(
                    stt[:, i, h, :], x_t[:, i, h * (M // 2):(h + 1) * (M // 2)]
                )
        mv = st["mv"] = self.sm_p.tile([128, CCH, 2], F32, tag="mv", bufs=4, name="mv")
        for i in range(CCH):
            nc.vector.bn_aggr(mv[:, i, :], stt[:, i, :, :])
        # trace(cov) = sum_c var_c, broadcast to all partitions via ones-matmul
        var_b = self.sm_p.tile([128, CCH], BF16, tag="var_b", name="var_b")
        nc.gpsimd.tensor_copy(var_b[:], mv[:, :, 1])
        t_ps = self.ps_sm.tile([128, CCH], F32, tag="col", name="sm")
        nc.tensor.matmul(t_ps[:], self.ones_b[:], var_b[:], start=True, stop=True)
        tco = self.sm_p.tile([128, 1], F32, tag="tco", name="tco")
        nc.vector.reduce_sum(out=tco[:], in_=t_ps[:], axis=AX)
        inv = self.sm_p.tile([128, 1], F32, tag="inv", name="inv")
        nc.vector.reciprocal(inv[:], tco[:])
        invM = st["invM"] = self.sm_p.tile([128, 1], F32, tag="invM", bufs=4, name="invM")
        nc.gpsimd.tensor_scalar_mul(invM[:], inv[:], 1.0 / M)
        sq = st["sq"] = self.sm_p.tile([128, 1], F32, tag="sq", bufs=4, name="sq")
        nc.scalar.sqrt(sq[:], tco[:])
        # mu as bf16 rows on partition 0: mupos [1,C] (lhsT slices), -M*mu (rhs)
        mn_ps = self.ps_sm.tile([1, C], F32, tag="row", name="mneg")
        for i in range(CCH):
            nc.tensor.transpose(
                mn_ps[0:1, i * 128:(i + 1) * 128], mv[:, i, 0:1], self.identf[:]
            )
        mupos = st["mupos"] = self.sm_p.tile(
            [1, C], BF16, tag="mupos", bufs=4, name="mupos"
        )
        nc.scalar.copy(mupos[:], mn_ps[:])
        muneg = st["muneg"] = self.sm_p.tile(
            [1, C], BF16, tag="muneg", bufs=4, name="muneg"
        )
        nc.scalar.mul(muneg[:], mn_ps[:], -float(M))

    # ---------- x transposes ----------
    def trans(self, s, j):
        nc, st = self.nc, self.S[s]
        if j == 0:
            st["xt"] = self.xt_p.tile([MC, MCH, C], BF16, tag="xt", name="xt")
        xt, x_t = st["xt"], st["x"]
        tp = self.ps_tr.tile([MC, CCH, 128], F32, tag="tr", name="tr")
        for i in range(CCH):
            nc.tensor.transpose(
                tp[:, i, :], x_t[:, i, j * MC:(j + 1) * MC], self.identf[:]
            )
        self._cp(xt[:, j, :], tp[:, :, :])

    # ---------- symmetric-matrix mirror ----------
    def _mirror(self, mat_t, acc=None):
        """Fill lower blocks of mat_t from the upper triangle.

        For dst chunk k: transpose blocks (i,k), i<k, into one PSUM tile, then
        one contiguous copy into mat_t[:, k, 0:k*128]. With acc, the copies
        run on Activation with accum_out into acc[:, k] (rowsums of the
        mirrored part).
        """
        nc = self.nc
        for k in range(1, CCH):
            mp = self.ps_mr.tile([128, 3, 128], BF16, tag="mr", name="mr")
            for i in range(k):
                nc.tensor.transpose(
                    mp[:, i, :], mat_t[:, i, k * 128:(k + 1) * 128], self.ident_b[:]
                )
            if acc is not None:
                nc.scalar.activation(
                    mat_t[:, k, 0:k * 128], mp[:, 0:k, :], COPYF,
                    accum_out=acc[:, k:k + 1],
                )
            else:
                self._cp(mat_t[:, k, 0:k * 128], mp[:, 0:k, :])

    # ---------- cov ----------
    def cov(self, s, i):
        nc, st = self.nc, self.S[s]
        if i == 0:
            st["w0"] = self.w_p.tile([128, CCH, C], BF16, tag="W", name="W0")
            st["acc_w0"] = self.sm_p.tile(
                [128, CCH], F32, tag="acc_w0", bufs=4, name="acc_w0"
            )
        xt, w0 = st["xt"], st["w0"]
        w = _wid(i)
        g = self.ps_mm.tile([128, C], F32, tag="mm", name="mm")
        for j in range(MCH):
            nc.tensor.matmul(
                g[:, 0:w], xt[:, j, i * 128:(i + 1) * 128], xt[:, j, C - w:],
                start=(j == 0), stop=False,
            )
        nc.tensor.matmul(
            g[:, 0:w], st["mupos"][0:1, i * 128:(i + 1) * 128],
            st["muneg"][0:1, C - w:],
            start=False, stop=True,
        )
        nc.scalar.activation(
            w0[:, i, C - w:], g[:, 0:w], COPYF,
            scale=st["invM"][:],
            accum_out=st["acc_w0"][:, i:i + 1],
        )

    def cov_mirror(self, s):
        st = self.S[s]
        st["acc_w0m"] = self.sm_p.tile(
            [128, CCH], F32, tag="acc_w0m", bufs=4, name="acc_w0m"
        )
        self._mirror(st["w0"], acc=st["acc_w0m"])

    # ---------- Newton-Schulz W iteration ----------
    def form_S(self, s, i, it):
        """S = W @ W, chunk i."""
        nc, st = self.nc, self.S[s]
        if i == 0:
            st["s"] = self.s_p.tile([128, CCH, C], BF16, tag="S", name="S")
            if it == 0:
                st["acc_s1"] = self.sm_p.tile(
                    [128, CCH], F32, tag="acc_s1", bufs=4, name="acc_s1"
                )
        wm, sm = st["w"], st["s"]
        w = _wid(i)
        ps = self.ps_mm.tile([128, C], F32, tag="mm", name="mm")
        for k in range(CCH):
            nc.tensor.matmul(
                ps[:, 0:w], wm[:, k, i * 128:(i + 1) * 128], wm[:, k, C - w:],
                start=(k == 0), stop=(k == CCH - 1),
            )
        if it == 0:
            nc.scalar.activation(
                sm[:, i, C - w:], ps[:, 0:w], COPYF,
                accum_out=st["acc_s1"][:, i:i + 1],
            )
        else:
            self._cp(sm[:, i, C - w:], ps[:, 0:w])

    def form_S_mirror(self, s, it):
        st = self.S[s]
        if it == 0:
            st["acc_s1m"] = self.sm_p.tile(
                [128, CCH], F32, tag="acc_s1m", bufs=4, name="acc_s1m"
            )
            self._mirror(st["s"], acc=st["acc_s1m"])
        else:
            self._mirror(st["s"])

    def form_W(self, s, i, it):
        """W' = (S @ W - 6 S + 9 W)/4, chunk i."""
        nc, st = self.nc, self.S[s]
        if i == 0:
            st["wn"] = self.w_p.tile([128, CCH, C], BF16, tag="W", name="Wn")
        wm, sm, wn = st["w"], st["s"], st["wn"]
        w = _wid(i)
        ps = self.ps_mm.tile([128, C], F32, tag="mm", name="mm")
        for k in range(CCH):
            nc.tensor.matmul(
                ps[:, 0:w], sm[:, k, i * 128:(i + 1) * 128], wm[:, k, C - w:],
                start=(k == 0), stop=False,
            )
        nc.tensor.matmul(
            ps[:, 0:w], self.diagm6[:], sm[:, i, C - w:], start=False, stop=False
        )
        nc.tensor.matmul(
            ps[:, 0:w], self.diag9[:], wm[:, i, C - w:], start=False, stop=True
        )
        self._cp(wn[:, i, C - w:], ps[:, 0:w], scale=0.25)

    def form_W_mirror(self, s):
        self._mirror(self.S[s]["wn"])

    # ---------- tail: row-chain matvecs ----------
    def tail_r2(self, s):
        """r2 = 1.5*rowsum(W0) - 0.5*rowsum(S1) as bf16 column [128, CCH]."""
        nc, st = self.nc, self.S[s]
        rs_w0 = self.sm_p.tile([128, CCH], F32, tag="rs_w0", name="rs_w0")
        nc.gpsimd.tensor_tensor(
            rs_w0[:, 1:], st["acc_w0"][:, 1:], st["acc_w0m"][:, 1:], op=ADD
        )
        nc.gpsimd.tensor_copy(rs_w0[:, 0:1], st["acc_w0"][:, 0:1])
        rs_s1 = self.sm_p.tile([128, CCH], F32, tag="rs_s1", name="rs_s1")
        nc.gpsimd.tensor_tensor(
            rs_s1[:, 1:], st["acc_s1"][:, 1:], st["acc_s1m"][:, 1:], op=ADD
        )
        nc.gpsimd.tensor_copy(rs_s1[:, 0:1], st["acc_s1"][:, 0:1])
        # r2 = 1.5*rs_w0 - 0.5*rs_s1 = 0.5*(3*rs_w0 - rs_s1); v15 = 1.5*r2
        t1 = self.sm_p.tile([128, CCH], F32, tag="t1", name="t1")
        nc.gpsimd.tensor_scalar_mul(t1[:], rs_w0[:], 3.0)
        nc.gpsimd.tensor_tensor(t1[:], t1[:], rs_s1[:], op=SUB)
        vc = self.sm_p.tile([128, CCH], BF16, tag="vc", bufs=4, name="vc")
        nc.gpsimd.tensor_scalar_mul(vc[:], t1[:], 0.5)
        v15 = self.sm_p.tile([128, CCH], F32, tag="v15", bufs=4, name="v15")
        nc.gpsimd.tensor_scalar_mul(v15[:], t1[:], 0.75)
        st["vc"], st["v15"] = vc, v15

    def tail_step(self, s, mat, last=False, save_r4=False, need_v15=True):
        """vc <- 1.5*vc - 0.5*(vc @ mat)."""
        nc, st = self.nc, self.S[s]
        mt = st[mat]
        pr = self.ps_sm.tile([1, C], F32, tag="row", name="row")
        for k in range(CCH):
            nc.tensor.matmul(
                pr[:], st["vc"][:, k:k + 1], mt[:, k, :],
                start=(k == 0), stop=(k == CCH - 1),
            )
        rr = self.sm_p.tile([1, C], BF16, tag="rr", bufs=3, name="rr")
        nc.scalar.mul(rr[:], pr[:], -0.5)
        tpc = self.ps_sm.tile([128, CCH, 2], BF16, tag="col", name="tpc")
        for k in range(CCH):
            nc.tensor.transpose(
                tpc[:, k, 0:1], rr[0:1, k * 128:(k + 1) * 128],
                self.ident_b[0:1, 0:1],
            )
        if last:
            # fs = (1.5*r4 - 0.5*(v @ mat)) * sq / C
            pre = self.sm_p.tile([128, CCH], F32, tag="pre", name="pre")
            nc.vector.tensor_tensor(pre[:], tpc[:, :, 0], st["v15_r4"][:], op=ADD)
            fs = st["fs"] = self.sm_p.tile([128, CCH], F32, tag="fs", bufs=4, name="fs")
            nc.gpsimd.tensor_scalar(
                fs[:], pre[:], st["sq"][:], 1.0 / C, op0=MULT, op1=MULT
            )
            return
        vn = self.sm_p.tile([128, CCH], BF16, tag="vc", bufs=4, name="vcn")
        nc.vector.tensor_tensor(vn[:], tpc[:, :, 0], st["v15"][:], op=ADD)
        st["vc"] = vn
        if not need_v15:
            return
        if save_r4:
            v15n = self.sm_p.tile([128, CCH], F32, tag="v15r4", bufs=2, name="v15r4")
            st["v15_r4"] = v15n
        else:
            v15n = self.sm_p.tile([128, CCH], F32, tag="v15", bufs=4, name="v15n")
        nc.gpsimd.tensor_scalar_mul(v15n[:], vn[:], 1.5)
        st["v15"] = v15n

    # ---------- final scale & output ----------
    def fin(self, s):
        nc, st = self.nc, self.S[s]
        x_t, fs = st["x"], st["fs"]
        for i in range(CCH):
            eng = (nc.vector, nc.gpsimd, nc.scalar, nc.gpsimd)[i]
            if eng is nc.scalar:
                eng.mul(x_t[:, i, :], x_t[:, i, :], fs[:, i:i + 1])
            else:
                eng.tensor_scalar_mul(x_t[:, i, :], x_t[:, i, :], fs[:, i:i + 1])
        nc.sync.dma_start(self.yr[s], x_t[:])
        st.clear()

    # ---------- braiding generators ----------
    def gen_transcov(self, pair):
        for s in pair:
            self.stats(s)
        for j in range(MCH):
            for s in pair:
                self.trans(s, j)
            yield
        for i in range(CCH):
            for s in pair:
                self.cov(s, i)
            yield
        for s in pair:
            self.cov_mirror(s)
            self.S[s]["w"] = self.S[s]["w0"]
            yield

    def gen_ns(self, pair):
        for it in range(3):
            for i in range(CCH):
                for s in pair:
                    self.form_S(s, i, it)
                yield
            for s in pair:
                self.form_S_mirror(s, it)
                yield
            for i in range(CCH):
                for s in pair:
                    self.form_W(s, i, it)
                yield
            for s in pair:
                self.form_W_mirror(s)
                st = self.S[s]
                st["w"] = st["wn"]
                st["w%d" % (it + 1)] = st["wn"]
                yield

    def gen_tail(self, s):
        self.tail_r2(s)
        yield
        self.tail_step(s, "w1")
        yield
        self.tail_step(s, "w2")
        yield
        self.tail_step(s, "w3", save_r4=True)
        yield
        self.tail_step(s, "w3")
        yield
        self.tail_step(s, "w3", need_v15=False)
        yield
        self.tail_step(s, "w3", last=True)

    @staticmethod
    def _round_robin(gens):
        done = [False] * len(gens)
        while not all(done):
            for gi, g in enumerate(gens):
                if not done[gi]:
                    try:
                        next(g)
                    except StopIteration:
                        done[gi] = True


def _emit(ctx, tc, x_ap, y_ap):
    em = _Emit(ctx, tc, x_ap, y_ap)
    for s in range(B_LOC):
        em.dma_in(s)
    em._round_robin([em.gen_transcov((0, 1))])
    em._round_robin([em.gen_ns((0, 1))])
    em._round_robin([em.gen_tail(0), em.gen_tail(1), em.gen_transcov((2, 3))])
    em.fin(0)
    em.fin(1)
    em._round_robin([em.gen_ns((2, 3))])
    em._round_robin([em.gen_tail(2), em.gen_tail(3)])
    em.fin(2)
    em.fin(3)


_NC_CACHE = {}


def _get_nc(reps: int = 1):
    if reps not in _NC_CACHE:
        nc = bacc.Bacc("TRN2", target_bir_lowering=False, debug=False)
        x_ap = nc.dram_tensor("x", [B_LOC, C, M], F32, kind="ExternalInput").ap()
        y_ap = nc.dram_tensor("y", [B_LOC, C, M], F32, kind="ExternalOutput").ap()
        with ExitStack() as ctx:
            tc = ctx.enter_context(tile.TileContext(nc))
            if reps > 1:
                with tc.For_i(0, reps, 1):
                    _emit(ctx, tc, x_ap, y_ap)
            else:
                _emit(ctx, tc, x_ap, y_ap)
        nc.compile()
        _NC_CACHE[reps] = nc
    return _NC_CACHE[reps]


def kernel(x: np.ndarray, _trace: bool = False):
    assert x.shape == (B, C, H, W), x.shape
    xs = np.ascontiguousarray(x.reshape(B, C, M), dtype=np.float32)
    nc = _get_nc()
    in_maps = [
        {"x": np.ascontiguousarray(xs[c * B_LOC:(c + 1) * B_LOC])}
        for c in range(N_CORES)
    ]
    res = run_bass_kernel_spmd(nc, in_maps, core_ids=list(range(N_CORES)), trace=_trace)
    y = np.concatenate([res.results[c]["y"] for c in range(N_CORES)], axis=0)
    out = y.reshape(B, C, H, W).astype(np.float32)
    if _trace:
        return out, res
    return out


# revision 14
# speedup vs baseline: 2.9542x; 1.9754x over previous
"""Trainium2 Bass kernel for nn_Covar_Attn (MPNCOV-style covariance pooling).

Per sample s (of 32): X = x[s] viewed [C=512, M=784]
  cov  = (X-mu) @ (X-mu)^T / M                  [512, 512]
  A    = cov / trace(cov)
  Ysqrt= Newton-Schulz(A, 5 iters) * sqrt(trace)
  w    = mean over rows of Ysqrt                [512]
  y[s] = w[:, None] * X

Sharding: pure data parallel, 4 samples per NeuronCore across 8 cores.

v2 design:
- All matmul operands bf16 (1 cyc/col at any width; FWL weight loads on HW).
- No centering pass: cov accumulated from raw x with a rank-1 -M*mu*mu^T
  matmul folded into the same PSUM; x stays raw so the final scale is one op.
- W-iteration Newton-Schulz: with W_k = Z_k Y_k (= A Z_k^2),
  W_{k+1} = (9 W - 6 W^2 + W^3)/4 needs 2 matrix products per iteration
  (vs 3 for the Y/Z form); the 9W and -6W^2 terms are accumulated into the
  W^3 PSUM via constant-diagonal matmuls.
- 1^T Ysqrt = 1^T An ZY0 ZY1 ZY2 ZY3 ZY4 with v@ZY_k = 1.5v - 0.5 v@W_k and
  v@W4 expanded into three extra W3 matvecs. The first two steps come free
  from accum_out rowsums of the An and S1 copy-outs; the remaining 6 matvec
  steps run as row-mode chains on the PE.
- All matrices symmetric: only upper-triangle chunk-rows are computed
  (true-triangle widths 512/384/256/128); lower blocks are mirrored with PE
  transposes and destination-contiguous batched copies.
- PSUM->SBUF copies alternate between Activation and DVE (gpsimd has no
  PSUM port); gpsimd takes SBUF-resident work (final scaling, combines).
"""

import numpy as np
from contextlib import ExitStack

import concourse.bass as bass
import concourse.mybir as mybir
import concourse.tile as tile
from concourse import bacc
from concourse.bass_utils import run_bass_kernel_spmd

N_CORES = 8
B, C, H, W = 32, 512, 28, 28
M = H * W            # 784
B_LOC = B // N_CORES  # 4 samples per core
CCH = C // 128       # 4 chunks of 128 rows
MCH = 7              # m chunks
MC = M // MCH        # 112

F32 = mybir.dt.float32
F32R = mybir.dt.float32r
BF16 = mybir.dt.bfloat16
MULT = mybir.AluOpType.mult
ADD = mybir.AluOpType.add
SUB = mybir.AluOpType.subtract
AX = mybir.AxisListType.X
COPYF = mybir.ActivationFunctionType.Copy


def _wid(i):
    # true-triangle width of chunk-row i (diag block + right part)
    return C - i * 128


def _fill_diag(nc, t, val):
    nc.gpsimd.memset(t[:], 0.0)
    nc.gpsimd.affine_select(
        out=t[:],
        in_=t[:],
        compare_op=mybir.AluOpType.not_equal,
        fill=val,
        base=0,
        pattern=[[-1, 128]],
        channel_multiplier=1,
    )


class _Emit:
    def __init__(self, ctx, tc, x_ap, y_ap):
        nc = self.nc = tc.nc
        self.tc = tc
        p = lambda name, bufs, **kw: ctx.enter_context(
            tc.tile_pool(name=name, bufs=bufs, **kw)
        )
        self.consts = p("consts", 1)
        self.xin_p = p("xin", 4)
        self.xt_p = p("xt", 2)
        self.w_p = p("wmat", 8)
        self.s_p = p("smat", 3)
        self.sm_p = p("sm", 2)
        self.ps_mm = p("psmm", 2, space="PSUM")
        self.ps_tr = p("pstr", 2, space="PSUM")
        self.ps_mr = p("psmr", 2, space="PSUM")
        self.ps_sm = p("pssm", 1, space="PSUM")

        # constants
        identf = self.identf = self.consts.tile([128, 128], F32, tag="identf", name="identf")
        _fill_diag(nc, identf, 1.0)
        self.ident_b = self.consts.tile([128, 128], BF16, tag="ident_b", name="ident_b")
        nc.vector.tensor_copy(self.ident_b[:], identf[:])
        d9 = self.consts.tile([128, 128], F32, tag="d9f", name="d9f")
        _fill_diag(nc, d9, 9.0)
        self.diag9 = self.consts.tile([128, 128], BF16, tag="diag9", name="diag9")
        nc.vector.tensor_copy(self.diag9[:], d9[:])
        dm6 = self.consts.tile([128, 128], F32, tag="dm6f", name="dm6f")
        _fill_diag(nc, dm6, -6.0)
        self.diagm6 = self.consts.tile([128, 128], BF16, tag="diagm6", name="diagm6")
        nc.vector.tensor_copy(self.diagm6[:], dm6[:])
        onesf = self.consts.tile([128, 128], F32, tag="onesf", name="onesf")
        nc.gpsimd.memset(onesf[:], 1.0)
        self.ones_b = self.consts.tile([128, 128], BF16, tag="ones_b", name="ones_b")
        nc.vector.tensor_copy(self.ones_b[:], onesf[:])

        self.xr = x_ap.rearrange("b (i p) m -> b p i m", p=128)
        self.yr = y_ap.rearrange("b (i p) m -> b p i m", p=128)
        self.S = [dict() for _ in range(B_LOC)]
        self._cp_i = 0

    def _cp(self, out, in_, scale=None):
        """PSUM->SBUF copy alternating between Activation and DVE."""
        use_act = (self._cp_i % 2 == 0)
        self._cp_i += 1
        if use_act:
            if scale is None:
                self.nc.scalar.copy(out, in_)
            else:
                self.nc.scalar.mul(out, in_, scale)
        else:
            if scale is None:
                self.nc.vector.tensor_copy(out, in_)
            else:
                self.nc.vector.tensor_scalar_mul(out, in_, scale)

    # ---------- load & stats ----------
    def dma_in(self, s):
        nc, st = self.nc, self.S[s]
        x_t = st["x"] = self.xin_p.tile([128, CCH, M], F32, tag="x", name="x")
        for i in range(CCH):
            nc.sync.dma_start(x_t[:, i, :], self.xr[s, :, i, :])

    def stats(self, s):
        nc, st = self.nc, self.S[s]
        x_t = st["x"]
        stt = self.sm_p.tile([128, CCH, 2, 6], F32, tag="st", name="st")
        for i in range(CCH):
            for h in range(2):
                nc.vector.bn_stats(
                    stt[:, i, h, :], x_t[:, i, h * (M // 2):(h + 1) * (M // 2)]
                )
        mv = st["mv"] = self.sm_p.tile([128, CCH, 2], F32, tag="mv", bufs=4, name="mv")
        for i in range(CCH):
            nc.vector.bn_aggr(mv[:, i, :], stt[:, i, :, :])
        # trace(cov) = sum_c var_c, broadcast to all partitions via ones-matmul
        var_b = self.sm_p.tile([128, CCH], BF16, tag="var_b", name="var_b")
        nc.gpsimd.tensor_copy(var_b[:], mv[:, :, 1])
        t_ps = self.ps_sm.tile([128, CCH], F32, tag="col", name="sm")
        nc.tensor.matmul(t_ps[:], self.ones_b[:], var_b[:], start=True, stop=True)
        tco = self.sm_p.tile([128, 1], F32, tag="tco", name="tco")
        nc.vector.reduce_sum(out=tco[:], in_=t_ps[:], axis=AX)
        inv = self.sm_p.tile([128, 1], F32, tag="inv", name="inv")
        nc.vector.reciprocal(inv[:], tco[:])
        invM = st["invM"] = self.sm_p.tile([128, 1], F32, tag="invM", bufs=4, name="invM")
        nc.gpsimd.tensor_scalar_mul(invM[:], inv[:], 1.0 / M)
        sq = st["sq"] = self.sm_p.tile([128, 1], F32, tag="sq", bufs=4, name="sq")
        nc.scalar.sqrt(sq[:], tco[:])
        # mu as bf16 rows on partition 0: mupos [1,C] (lhsT slices), -M*mu (rhs)
        mn_ps = self.ps_sm.tile([1, C], F32, tag="row", name="mneg")
        for i in range(CCH):
            nc.tensor.transpose(
                mn_ps[0:1, i * 128:(i + 1) * 128], mv[:, i, 0:1], self.identf[:]
            )
        mupos = st["mupos"] = self.sm_p.tile(
            [1, C], BF16, tag="mupos", bufs=4, name="mupos"
        )
        nc.scalar.copy(mupos[:], mn_ps[:])
        muneg = st["muneg"] = self.sm_p.tile(
            [1, C], BF16, tag="muneg", bufs=4, name="muneg"
        )
        nc.scalar.mul(muneg[:], mn_ps[:], -float(M))

    # ---------- x transposes ----------
    def trans(self, s, j):
        nc, st = self.nc, self.S[s]
        if j == 0:
            st["xt"] = self.xt_p.tile([MC, MCH, C], BF16, tag="xt", name="xt")
        xt, x_t = st["xt"], st["x"]
        tp = self.ps_tr.tile([MC, CCH, 128], F32, tag="tr", name="tr")
        for i in range(CCH):
            nc.tensor.transpose(
                tp[:, i, :], x_t[:, i, j * MC:(j + 1) * MC], self.identf[:]
            )
        self._cp(xt[:, j, :], tp[:, :, :])

    # ---------- symmetric-matrix mirror ----------
    def _mirror(self, mat_t, acc=None):
        """Fill lower blocks of mat_t from the upper triangle.

        For dst chunk k: transpose blocks (i,k), i<k, into one PSUM tile, then
        one contiguous copy into mat_t[:, k, 0:k*128]. With acc, the copies
        run on Activation with accum_out into acc[:, k] (rowsums of the
        mirrored part).
        """
        nc = self.nc
        for k in range(1, CCH):
            mp = self.ps_mr.tile([128, 3, 128], BF16, tag="mr", name="mr")
            for i in range(k):
                nc.tensor.transpose(
                    mp[:, i, :], mat_t[:, i, k * 128:(k + 1) * 128], self.ident_b[:]
                )
            if acc is not None:
                nc.scalar.activation(
                    mat_t[:, k, 0:k * 128], mp[:, 0:k, :], COPYF,
                    accum_out=acc[:, k:k + 1],
                )
            else:
                self._cp(mat_t[:, k, 0:k * 128], mp[:, 0:k, :])

    # ---------- cov ----------
    def cov(self, s, i):
        nc, st = self.nc, self.S[s]
        if i == 0:
            st["w0"] = self.w_p.tile([128, CCH, C], BF16, tag="W", name="W0")
            st["acc_w0"] = self.sm_p.tile(
                [128, CCH], F32, tag="acc_w0", bufs=4, name="acc_w0"
            )
        xt, w0 = st["xt"], st["w0"]
        w = _wid(i)
        g = self.ps_mm.tile([128, C], F32, tag="mm", name="mm")
        for j in range(MCH):
            nc.tensor.matmul(
                g[:, 0:w], xt[:, j, i * 128:(i + 1) * 128], xt[:, j, C - w:],
                start=(j == 0), stop=False,
            )
        nc.tensor.matmul(
            g[:, 0:w], st["mupos"][0:1, i * 128:(i + 1) * 128],
            st["muneg"][0:1, C - w:],
            start=False, stop=True,
        )
        nc.scalar.activation(
            w0[:, i, C - w:], g[:, 0:w], COPYF,
            scale=st["invM"][:],
            accum_out=st["acc_w0"][:, i:i + 1],
        )

    def cov_mirror(self, s):
        st = self.S[s]
        st["acc_w0m"] = self.sm_p.tile(
            [128, CCH], F32, tag="acc_w0m", bufs=4, name="acc_w0m"
        )
        self._mirror(st["w0"], acc=st["acc_w0m"])

    # ---------- Newton-Schulz W iteration ----------
    def form_S(self, s, i, it):
        """S = W @ W, chunk i."""
        nc, st = self.nc, self.S[s]
        if i == 0:
            st["s"] = self.s_p.tile([128, CCH, C], BF16, tag="S", name="S")
            if it == 0:
                st["acc_s1"] = self.sm_p.tile(
                    [128, CCH], F32, tag="acc_s1", bufs=4, name="acc_s1"
                )
        wm, sm = st["w"], st["s"]
        w = _wid(i)
        ps = self.ps_mm.tile([128, C], F32, tag="mm", name="mm")
        for k in range(CCH):
            nc.tensor.matmul(
                ps[:, 0:w], wm[:, k, i * 128:(i + 1) * 128], wm[:, k, C - w:],
                start=(k == 0), stop=(k == CCH - 1),
            )
        if it == 0:
            nc.scalar.activation(
                sm[:, i, C - w:], ps[:, 0:w], COPYF,
                accum_out=st["acc_s1"][:, i:i + 1],
            )
        else:
            self._cp(sm[:, i, C - w:], ps[:, 0:w])

    def form_S_mirror(self, s, it):
        st = self.S[s]
        if it == 0:
            st["acc_s1m"] = self.sm_p.tile(
                [128, CCH], F32, tag="acc_s1m", bufs=4, name="acc_s1m"
            )
            self._mirror(st["s"], acc=st["acc_s1m"])
        else:
            self._mirror(st["s"])

    def form_W_pair(self, pair, i):
        """W' = (S @ W - 6 S + 9 W)/4, chunk i, both samples of the pair.

        The S@W parts run per sample; the diag-const matmuls are grouped so
        consecutive PE instructions share the same stationary operand
        (one LdWeights per const per chunk instead of per sample)."""
        nc = self.nc
        pss = {}
        for s in pair:
            st = self.S[s]
            if i == 0:
                st["wn"] = self.w_p.tile([128, CCH, C], BF16, tag="W", name="Wn")
            wm, sm = st["w"], st["s"]
            w = _wid(i)
            ps = pss[s] = self.ps_mm.tile([128, C], F32, tag="mm", name="mm")
            for k in range(CCH):
                nc.tensor.matmul(
                    ps[:, 0:w], sm[:, k, i * 128:(i + 1) * 128], wm[:, k, C - w:],
                    start=(k == 0), stop=False,
                )
        w = _wid(i)
        for s in pair:
            nc.tensor.matmul(
                pss[s][:, 0:w], self.diagm6[:], self.S[s]["s"][:, i, C - w:],
                start=False, stop=False,
            )
        for s in pair:
            nc.tensor.matmul(
                pss[s][:, 0:w], self.diag9[:], self.S[s]["w"][:, i, C - w:],
                start=False, stop=True,
            )
        for s in pair:
            self._cp(self.S[s]["wn"][:, i, C - w:], pss[s][:, 0:w], scale=0.25)

    def form_W_mirror(self, s):
        self._mirror(self.S[s]["wn"])

    # ---------- tail: row-chain matvecs ----------
    def tail_r2(self, s):
        """r2 = 1.5*rowsum(W0) - 0.5*rowsum(S1) as bf16 column [128, CCH]."""
        nc, st = self.nc, self.S[s]
        rs_w0 = self.sm_p.tile([128, CCH], F32, tag="rs_w0", name="rs_w0")
        nc.gpsimd.tensor_tensor(
            rs_w0[:, 1:], st["acc_w0"][:, 1:], st["acc_w0m"][:, 1:], op=ADD
        )
        nc.gpsimd.tensor_copy(rs_w0[:, 0:1], st["acc_w0"][:, 0:1])
        rs_s1 = self.sm_p.tile([128, CCH], F32, tag="rs_s1", name="rs_s1")
        nc.gpsimd.tensor_tensor(
            rs_s1[:, 1:], st["acc_s1"][:, 1:], st["acc_s1m"][:, 1:], op=ADD
        )
        nc.gpsimd.tensor_copy(rs_s1[:, 0:1], st["acc_s1"][:, 0:1])
        # r2 = 1.5*rs_w0 - 0.5*rs_s1 = 0.5*(3*rs_w0 - rs_s1); v15 = 1.5*r2
        t1 = self.sm_p.tile([128, CCH], F32, tag="t1", name="t1")
        nc.gpsimd.tensor_scalar_mul(t1[:], rs_w0[:], 3.0)
        nc.gpsimd.tensor_tensor(t1[:], t1[:], rs_s1[:], op=SUB)
        vc = self.sm_p.tile([128, CCH], BF16, tag="vc", bufs=4, name="vc")
        nc.gpsimd.tensor_scalar_mul(vc[:], t1[:], 0.5)
        v15 = self.sm_p.tile([128, CCH], F32, tag="v15", bufs=4, name="v15")
        nc.gpsimd.tensor_scalar_mul(v15[:], t1[:], 0.75)
        st["vc"], st["v15"] = vc, v15

    def tail_step(self, s, mat, last=False, save_r4=False, need_v15=True):
        """vc <- 1.5*vc - 0.5*(vc @ mat)."""
        nc, st = self.nc, self.S[s]
        mt = st[mat]
        pr = self.ps_sm.tile([1, C], F32, tag="row", name="row")
        for k in range(CCH):
            nc.tensor.matmul(
                pr[:], st["vc"][:, k:k + 1], mt[:, k, :],
                start=(k == 0), stop=(k == CCH - 1),
            )
        rr = self.sm_p.tile([1, C], BF16, tag="rr", bufs=3, name="rr")
        if s % 2 == 0:
            nc.scalar.mul(rr[:], pr[:], -0.5)
        else:
            nc.vector.tensor_scalar_mul(rr[:], pr[:], -0.5)
        tpc = self.ps_sm.tile([128, CCH, 2], BF16, tag="col", name="tpc")
        for k in range(CCH):
            nc.tensor.transpose(
                tpc[:, k, 0:1], rr[0:1, k * 128:(k + 1) * 128],
                self.ident_b[0:1, 0:1],
            )
        if last:
            # fs = (1.5*r4 - 0.5*(v @ mat)) * sq / C
            pre = self.sm_p.tile([128, CCH], F32, tag="pre", name="pre")
            nc.vector.tensor_tensor(pre[:], tpc[:, :, 0], st["v15_r4"][:], op=ADD)
            fs = st["fs"] = self.sm_p.tile([128, CCH], F32, tag="fs", bufs=4, name="fs")
            nc.gpsimd.tensor_scalar(
                fs[:], pre[:], st["sq"][:], 1.0 / C, op0=MULT, op1=MULT
            )
            return
        vn = self.sm_p.tile([128, CCH], BF16, tag="vc", bufs=4, name="vcn")
        nc.vector.tensor_tensor(vn[:], tpc[:, :, 0], st["v15"][:], op=ADD)
        st["vc"] = vn
        if not need_v15:
            return
        if save_r4:
            v15n = self.sm_p.tile([128, CCH], F32, tag="v15r4", bufs=2, name="v15r4")
            st["v15_r4"] = v15n
        else:
            v15n = self.sm_p.tile([128, CCH], F32, tag="v15", bufs=4, name="v15n")
        nc.vector.tensor_scalar_mul(v15n[:], vn[:], 1.5)
        st["v15"] = v15n

    # ---------- final scale & output ----------
    def fin(self, s):
        nc, st = self.nc, self.S[s]
        x_t, fs = st["x"], st["fs"]
        for i in range(CCH):
            eng = (nc.vector, nc.gpsimd, nc.scalar, nc.gpsimd)[i]
            if eng is nc.scalar:
                eng.mul(x_t[:, i, :], x_t[:, i, :], fs[:, i:i + 1])
            else:
                eng.tensor_scalar_mul(x_t[:, i, :], x_t[:, i, :], fs[:, i:i + 1])
            nc.sync.dma_start(self.yr[s, :, i, :], x_t[:, i, :])
        st.clear()

    # ---------- braiding generators ----------
    def gen_transcov(self, pair):
        for s in pair:
            self.stats(s)
        for j in range(MCH):
            for s in pair:
                self.trans(s, j)
            yield
        for i in range(CCH):
            for s in pair:
                self.cov(s, i)
            yield
        for s in pair:
            self.cov_mirror(s)
            self.S[s]["w"] = self.S[s]["w0"]
            yield

    def gen_ns(self, pair):
        for it in range(3):
            for i in range(CCH):
                for s in pair:
                    self.form_S(s, i, it)
                yield
            for s in pair:
                self.form_S_mirror(s, it)
                yield
            for i in range(CCH):
                self.form_W_pair(pair, i)
                yield
            for s in pair:
                self.form_W_mirror(s)
                st = self.S[s]
                st["w"] = st["wn"]
                st["w%d" % (it + 1)] = st["wn"]
                yield

    def gen_tail(self, s):
        self.tail_r2(s)
        yield
        self.tail_step(s, "w1")
        yield
        self.tail_step(s, "w2")
        yield
        self.tail_step(s, "w3", save_r4=True)
        yield
        self.tail_step(s, "w3")
        yield
        self.tail_step(s, "w3", need_v15=False)
        yield
        self.tail_step(s, "w3", last=True)

    @staticmethod
    def _round_robin(gens):
        done = [False] * len(gens)
        while not all(done):
            for gi, g in enumerate(gens):
                if not done[gi]:
                    try:
                        next(g)
                    except StopIteration:
                        done[gi] = True


def _emit(ctx, tc, x_ap, y_ap):
    em = _Emit(ctx, tc, x_ap, y_ap)
    for s in range(B_LOC):
        em.dma_in(s)
    em._round_robin([em.gen_transcov((0, 1))])
    em._round_robin([em.gen_ns((0, 1)), em.gen_transcov((2, 3))])
    em._round_robin([em.gen_tail(0), em.gen_tail(1), em.gen_ns((2, 3))])
    em.fin(0)
    em.fin(1)
    em._round_robin([em.gen_tail(2), em.gen_tail(3)])
    em.fin(2)
    em.fin(3)


_NC_CACHE = {}


def _get_nc(reps: int = 1):
    if reps not in _NC_CACHE:
        nc = bacc.Bacc("TRN2", target_bir_lowering=False, debug=False)
        x_ap = nc.dram_tensor("x", [B_LOC, C, M], F32, kind="ExternalInput").ap()
        y_ap = nc.dram_tensor("y", [B_LOC, C, M], F32, kind="ExternalOutput").ap()
        with ExitStack() as ctx:
            tc = ctx.enter_context(tile.TileContext(nc))
            if reps > 1:
                with tc.For_i(0, reps, 1):
                    _emit(ctx, tc, x_ap, y_ap)
            else:
                _emit(ctx, tc, x_ap, y_ap)
        nc.compile()
        _NC_CACHE[reps] = nc
    return _NC_CACHE[reps]


def kernel(x: np.ndarray, _trace: bool = False):
    assert x.shape == (B, C, H, W), x.shape
    xs = np.ascontiguousarray(x.reshape(B, C, M), dtype=np.float32)
    nc = _get_nc()
    in_maps = [
        {"x": np.ascontiguousarray(xs[c * B_LOC:(c + 1) * B_LOC])}
        for c in range(N_CORES)
    ]
    res = run_bass_kernel_spmd(nc, in_maps, core_ids=list(range(N_CORES)), trace=_trace)
    y = np.concatenate([res.results[c]["y"] for c in range(N_CORES)], axis=0)
    out = y.reshape(B, C, H, W).astype(np.float32)
    if _trace:
        return out, res
    return out
